# revision 7
# baseline (speedup 1.0000x reference)
"""MoE router (NoisyKGate) Trainium2 Bass kernel.

Computes, for x [B,S,D], W [D,E], b [E], k=8:
    s = sigmoid(x @ W + b)            # [B,S,E]
    g_i, idx = top_k(s, 8)            # [B,S,8]
    g = g_i / sum(g_i)
Returns (g, idx_int32, s).

Strategy: data-parallel over tokens across 8 NeuronCores; each core gets a
2048-token shard. The matmul runs as an fp16 high/low 3-pass decomposition:
on the host, x = xh + 2^-12 * xl and W = wh + 2^-12 * wl with xh/xl/wh/wl all
fp16. The PE multiplies fp16 exactly (11x11-bit mantissas fit the FP22
datapath) and accumulates fp32, so
    x@W ~= xh@wh + 2^-12 * (xh@wl + xl@wh)        (error ~2^-22 relative)
which beats the HW fp32 4-pass in accuracy while streaming at 1 cycle/row.

Layout: W-chunks are the stationary operand; x arrives host-packed per token
group in the exact SBUF layout [partition, chunk, hi/lo, token] so each
group is one fully-contiguous DMA. Matmuls are column-tiled: two D-chunks
run concurrently on PE column groups (0,0)/(0,64), accumulating into PSUM
partitions 0:64 / 64:128; the chunk-parity halves are summed after the PE
transpose back to token-major layout, where the expert bias is added; ACT
applies sigmoid; DVE Max8/MaxIndex8 produce the top-8 values+indices; a
batched reciprocal-normalize finishes. Groups have descending token counts
so the tail (last group's PE + post-processing after the final DMA byte
lands) is short; the e2e span is within ~10% of the input-DMA roofline.
"""

import os

import numpy as np

B, S, D, E, TOPK = 4, 4096, 4096, 64, 8
N_CORES = 8
P = 128
TOK_PER_CORE = (B * S) // N_CORES  # 2048
KD = D // P  # 32 contraction chunks
CHUNKS = TOK_PER_CORE // P  # 16 token column-chunks per core

# token-group sizes (in 128-token tiles); descending so the pipeline tail
# after the final input DMA is minimal
GROUP_TILES = (4, 4, 4, 2, 1, 1)
assert sum(GROUP_TILES) == CHUNKS

LO_SCALE = float(2**-12)

_CACHE = {}
LAST_RESULTS = None


def _build_kernel(d=D, group_tiles=GROUP_TILES):
    import concourse.bacc as bacc
    import concourse.mybir as mybir
    from concourse.masks import make_identity
    from concourse.tile import TileContext

    f32 = mybir.dt.float32
    f16 = mybir.dt.float16
    kd = d // P
    chunks = sum(group_tiles)
    tok_per_core = chunks * P
    pairs = kd // 2

    nc = bacc.Bacc("TRN2", target_bir_lowering=False, debug=False)

    n_x = tok_per_core * d * 2  # fp16 elements across hi+lo
    xc_d = nc.dram_tensor("xc", [n_x], f16, kind="ExternalInput")
    wh_d = nc.dram_tensor("wh", [P, kd, E], f16, kind="ExternalInput")
    wl_d = nc.dram_tensor("wl", [P, kd, E], f16, kind="ExternalInput")
    br_d = nc.dram_tensor("br", [P, E], f32, kind="ExternalInput")
    s_out = nc.dram_tensor("s_out", [tok_per_core, E], f32, kind="ExternalOutput")
    g_out = nc.dram_tensor("g_out", [tok_per_core, TOPK], f32, kind="ExternalOutput")
    i_out = nc.dram_tensor(
        "i_out", [tok_per_core, TOPK], mybir.dt.int32, kind="ExternalOutput"
    )

    # token t = p*chunks + c lives at SBUF partition p, column-chunk c
    s_view = s_out.ap().rearrange("(p c) e -> p c e", p=P)
    g_view = g_out.ap().rearrange("(p c) k -> p c k", p=P)
    i_view = i_out.ap().rearrange("(p c) k -> p c k", p=P)
    xc_ap = xc_d.ap()

    with TileContext(nc) as tc:
        with (
            tc.tile_pool(name="const", bufs=1) as cpool,
            tc.tile_pool(name="xq", bufs=2) as xpool,
            tc.tile_pool(name="psum", bufs=2, space="PSUM") as ppool,
            tc.tile_pool(name="post", bufs=2) as spool,
        ):
            wh_sb = cpool.tile([P, kd, E], f16)
            nc.sync.dma_start(wh_sb[:], wh_d.ap())
            wl_sb = cpool.tile([P, kd, E], f16)
            nc.sync.dma_start(wl_sb[:], wl_d.ap())
            br_sb = cpool.tile([P, E], f32)
            nc.sync.dma_start(br_sb[:], br_d.ap())
            ident = cpool.tile([P, P], f32)
            make_identity(nc, ident[:])

            max_tiles = max(group_tiles)
            x_off = 0
            c0 = 0
            for q, g_tiles in enumerate(group_tiles):
                n_tok = g_tiles * P
                g_elems = P * kd * 2 * n_tok
                xsrc = xc_ap[x_off : x_off + g_elems].rearrange(
                    "(p ko h t) -> p ko h t", p=P, ko=kd, h=2, t=n_tok
                )
                xq = xpool.tile([P, kd, 2, max_tiles * P], f16, tag="xq")
                # split big group loads so the first matmuls start earlier
                n_dma = 2 if g_tiles >= 4 else 1
                step = kd // n_dma
                for j in range(n_dma):
                    nc.sync.dma_start(
                        xq[:, j * step : (j + 1) * step, :, :n_tok],
                        xsrc[:, j * step : (j + 1) * step],
                    )

                psA_t = ppool.tile([P, max_tiles * P], f32, tag="psA", name=f"psA_{q}")
                psB_t = ppool.tile([P, max_tiles * P], f32, tag="psB", name=f"psB_{q}")
                psA, psB = psA_t[:, :n_tok], psB_t[:, :n_tok]
                for ko2 in range(pairs):
                    k0, k1 = 2 * ko2, 2 * ko2 + 1
                    first = ko2 == 0
                    last = ko2 == pairs - 1
                    xh0, xl0 = xq[:, k0, 0, :n_tok], xq[:, k0, 1, :n_tok]
                    xh1, xl1 = xq[:, k1, 0, :n_tok], xq[:, k1, 1, :n_tok]
                    nc.tensor.matmul(
                        psA[0:64, :], wh_sb[:, k0, :], xh0,
                        tile_position=(0, 0), start=first, stop=last,
                        skip_group_check=True,
                    )
                    nc.tensor.matmul(
                        psA[64:128, :], wh_sb[:, k1, :], xh1,
                        tile_position=(0, 64), start=first, stop=last,
                        skip_group_check=True,
                    )
                    nc.tensor.matmul(
                        psB[0:64, :], wh_sb[:, k0, :], xl0,
                        tile_position=(0, 0), start=first, stop=False,
                        skip_group_check=True,
                    )
                    nc.tensor.matmul(
                        psB[64:128, :], wh_sb[:, k1, :], xl1,
                        tile_position=(0, 64), start=first, stop=False,
                        skip_group_check=True,
                    )
                    nc.tensor.matmul(
                        psB[0:64, :], wl_sb[:, k0, :], xh0,
                        tile_position=(0, 0), start=False, stop=last,
                        skip_group_check=True,
                    )
                    nc.tensor.matmul(
                        psB[64:128, :], wl_sb[:, k1, :], xh1,
                        tile_position=(0, 64), start=False, stop=last,
                        skip_group_check=True,
                    )

                # partial logits (chunk-parity halves stacked on partitions):
                # logitTT = psA + 2^-12 * psB    [128, n_tok]
                psA_sb = spool.tile([P, max_tiles * P], f32, tag="psA_sb")
                nc.vector.tensor_copy(psA_sb[:, :n_tok], psA)
                logitTT = spool.tile([P, max_tiles * P], f32, tag="logitTT")
                nc.vector.scalar_tensor_tensor(
                    logitTT[:, :n_tok], psB, LO_SCALE, psA_sb[:, :n_tok],
                    op0=mybir.AluOpType.mult, op1=mybir.AluOpType.add,
                )
                # transpose to token-major: psT [128 tok, (half, 64 expert)]
                psT_t = ppool.tile([P, max_tiles * P], f32, tag="psT", name=f"psT_{q}")
                for i in range(g_tiles):
                    nc.tensor.transpose(
                        psT_t[:, i * P : (i + 1) * P],
                        logitTT[:, i * P : (i + 1) * P],
                        ident[:],
                    )
                cc = spool.tile([P, max_tiles, P], f32, tag="cc")
                nc.vector.tensor_copy(
                    cc[:, :g_tiles],
                    psT_t[:, : g_tiles * P].rearrange("p (c he) -> p c he", c=g_tiles),
                )
                # sum the parity halves, add bias, sigmoid
                logit_q = spool.tile([P, max_tiles, E], f32, tag="logit_q")
                nc.vector.tensor_add(
                    logit_q[:, :g_tiles], cc[:, :g_tiles, 0:E], cc[:, :g_tiles, E:P]
                )
                nc.vector.tensor_add(
                    logit_q[:, :g_tiles],
                    logit_q[:, :g_tiles],
                    br_sb[:, None, :].to_broadcast([P, g_tiles, E]),
                )
                s_q = spool.tile([P, max_tiles, E], f32, tag="s_q")
                nc.scalar.activation(
                    s_q[:, :g_tiles], logit_q[:, :g_tiles],
                    mybir.ActivationFunctionType.Sigmoid,
                )

                # top-8 + normalize
                gmax = spool.tile([P, max_tiles, TOPK], f32, tag="gmax")
                gidx = spool.tile([P, max_tiles, TOPK], mybir.dt.uint32, tag="gidx")
                gsum = spool.tile([P, max_tiles], f32, tag="gsum")
                grec = spool.tile([P, max_tiles], f32, tag="grec")
                gnrm = spool.tile([P, max_tiles, TOPK], f32, tag="gnrm")
                for i in range(g_tiles):
                    nc.vector.max(out=gmax[:, i], in_=s_q[:, i])
                    nc.vector.max_index(
                        out=gidx[:, i], in_max=gmax[:, i], in_values=s_q[:, i]
                    )
                nc.vector.reduce_sum(
                    gsum[:, :g_tiles], gmax[:, :g_tiles], axis=mybir.AxisListType.X
                )
                nc.vector.reciprocal(grec[:, :g_tiles], gsum[:, :g_tiles])
                nc.vector.tensor_mul(
                    gnrm[:, :g_tiles],
                    gmax[:, :g_tiles],
                    grec[:, :g_tiles, None].to_broadcast([P, g_tiles, TOPK]),
                )

                c1 = c0 + g_tiles
                # outputs ride the ACT HWDGE ring so they never queue ahead
                # of the next group's input load on the Sync ring
                nc.scalar.dma_start(s_view[:, c0:c1, :], s_q[:, :g_tiles])
                nc.scalar.dma_start(g_view[:, c0:c1, :], gnrm[:, :g_tiles])
                nc.scalar.dma_start(
                    i_view[:, c0:c1, :], gidx[:, :g_tiles].bitcast(mybir.dt.int32)
                )
                c0 = c1
                x_off += g_elems

    nc.compile()
    return nc


def _get_nc():
    key = "main"
    if key not in _CACHE:
        _CACHE[key] = _build_kernel()
    return _CACHE[key]


def _split_hl(a32):
    """a32 (fp32) -> (hi fp16, lo fp16) with a32 ~= hi + 2^-12 * lo."""
    hi = a32.astype(np.float16)
    lo = ((a32 - hi.astype(np.float32)) * 4096.0).astype(np.float16)
    return hi, lo


def _pack_x(x, group_tiles, chunks, kd):
    """x [cores, tok_per_core, d] fp32 -> packed fp16 [cores, n_x].

    Group g covers token column-chunks [c0, c0+tiles); within it, matmul
    column i*128+pp holds token pp*chunks + c0 + i; layout per group is
    [p, ko, h, i, pp] flattened, groups concatenated.
    """
    n_cores = x.shape[0]
    xh, xl = _split_hl(x.reshape(n_cores, P, chunks, kd, P))
    parts = []
    c0 = 0
    for g_tiles in group_tiles:
        # [core, pp, tiles, ko, p] -> [core, p, ko, tiles, pp]
        gh = xh[:, :, c0 : c0 + g_tiles].transpose(0, 4, 3, 2, 1)
        gl = xl[:, :, c0 : c0 + g_tiles].transpose(0, 4, 3, 2, 1)
        blk = np.empty((n_cores, P, kd, 2, g_tiles, P), np.float16)
        blk[:, :, :, 0] = gh
        blk[:, :, :, 1] = gl
        parts.append(blk.reshape(n_cores, -1))
        c0 += g_tiles
    return np.concatenate(parts, axis=1)


def _prep_inputs(x, W, b):
    x = np.asarray(x, dtype=np.float32)
    W = np.ascontiguousarray(np.asarray(W, dtype=np.float32))
    b = np.ascontiguousarray(np.asarray(b, dtype=np.float32))

    xc = _pack_x(x.reshape(N_CORES, TOK_PER_CORE, D), GROUP_TILES, CHUNKS, KD)

    wh, wl = _split_hl(W)
    wh_sb = np.ascontiguousarray(wh.reshape(KD, P, E).transpose(1, 0, 2))
    wl_sb = np.ascontiguousarray(wl.reshape(KD, P, E).transpose(1, 0, 2))
    br = np.ascontiguousarray(np.broadcast_to(b.reshape(1, E), (P, E)))

    in_maps = []
    for j in range(N_CORES):
        in_maps.append({"xc": xc[j], "wh": wh_sb, "wl": wl_sb, "br": br})
    return in_maps


def kernel(x, W, b, k):
    global LAST_RESULTS
    from concourse.bass_utils import run_bass_kernel_spmd

    k = int(np.asarray(k))
    assert k == TOPK, f"kernel hardcodes k=8, got {k}"
    assert tuple(np.asarray(x).shape) == (B, S, D)

    nc = _get_nc()
    in_maps = _prep_inputs(x, W, b)

    res = run_bass_kernel_spmd(
        nc,
        in_maps,
        core_ids=list(range(N_CORES)),
        trace=bool(int(os.environ.get("KERNEL_TRACE", "0"))),
    )
    LAST_RESULTS = res

    g = np.concatenate([r["g_out"] for r in res.results], axis=0)
    idx = np.concatenate([r["i_out"] for r in res.results], axis=0)
    s = np.concatenate([r["s_out"] for r in res.results], axis=0)
    return (
        g.reshape(B, S, TOPK),
        idx.reshape(B, S, TOPK).astype(np.int32),
        s.reshape(B, S, E),
    )


# revision 8
# speedup vs baseline: 1.1798x; 1.1798x over previous
"""MoE router (NoisyKGate) Trainium2 Bass kernel.

Computes, for x [B,S,D], W [D,E], b [E], k=8:
    s = sigmoid(x @ W + b)            # [B,S,E]
    g_i, idx = top_k(s, 8)            # [B,S,8]
    g = g_i / sum(g_i)
Returns (g, idx_int32, s).

Strategy: data-parallel over tokens across 8 NeuronCores; each core gets a
2048-token shard. The matmul runs as an fp16 high/low 3-pass decomposition:
on the host, x = xh + 2^-12 * xl and W = wh + 2^-12 * wl with xh/xl/wh/wl all
fp16. The PE multiplies fp16 exactly (11x11-bit mantissas fit the FP22
datapath) and accumulates fp32, so
    x@W ~= xh@wh + 2^-12 * (xh@wl + xl@wh)        (error ~2^-22 relative)
which beats the HW fp32 4-pass in accuracy while streaming at 1 cycle/row.

Layout: W-chunks are the stationary operand; x arrives host-packed per token
group in the exact SBUF layout [partition, chunk, hi/lo, token] so each
group is one fully-contiguous DMA. Matmuls are column-tiled: two D-chunks
run concurrently on PE column groups (0,0)/(0,64), accumulating into PSUM
partitions 0:64 / 64:128; the chunk-parity halves are summed after the PE
transpose back to token-major layout, where the expert bias is added; ACT
applies sigmoid; DVE Max8/MaxIndex8 produce the top-8 values+indices; a
batched reciprocal-normalize finishes. Groups have descending token counts
so the tail (last group's PE + post-processing after the final DMA byte
lands) is short; the e2e span is within ~10% of the input-DMA roofline.
"""

import os

import numpy as np

B, S, D, E, TOPK = 4, 4096, 4096, 64, 8
N_CORES = 8
P = 128
TOK_PER_CORE = (B * S) // N_CORES  # 2048
KD = D // P  # 32 contraction chunks
CHUNKS = TOK_PER_CORE // P  # 16 token column-chunks per core

# token-group sizes (in 128-token tiles); descending so the pipeline tail
# after the final input DMA is minimal
GROUP_TILES = (4, 4, 4, 2, 1, 1)
assert sum(GROUP_TILES) == CHUNKS

LO_SCALE = float(2**-12)

_CACHE = {}
LAST_RESULTS = None


def _build_kernel(d=D, group_tiles=GROUP_TILES):
    import concourse.bacc as bacc
    import concourse.mybir as mybir
    from concourse.masks import make_identity
    from concourse.tile import TileContext

    f32 = mybir.dt.float32
    f16 = mybir.dt.float16
    kd = d // P
    chunks = sum(group_tiles)
    tok_per_core = chunks * P
    pairs = kd // 2

    nc = bacc.Bacc("TRN2", target_bir_lowering=False, debug=False)

    xc_ds = [
        nc.dram_tensor(f"xc{q}", [P, kd, 2, gt * P], f16, kind="ExternalInput")
        for q, gt in enumerate(group_tiles)
    ]
    wh_d = nc.dram_tensor("wh", [P, kd, E], f16, kind="ExternalInput")
    wl_d = nc.dram_tensor("wl", [P, kd, E], f16, kind="ExternalInput")
    br_d = nc.dram_tensor("br", [P, E], f32, kind="ExternalInput")
    s_out = nc.dram_tensor("s_out", [tok_per_core, E], f32, kind="ExternalOutput")
    g_out = nc.dram_tensor("g_out", [tok_per_core, TOPK], f32, kind="ExternalOutput")
    i_out = nc.dram_tensor(
        "i_out", [tok_per_core, TOPK], mybir.dt.int32, kind="ExternalOutput"
    )

    # token t = p*chunks + c lives at SBUF partition p, column-chunk c
    s_view = s_out.ap().rearrange("(p c) e -> p c e", p=P)
    g_view = g_out.ap().rearrange("(p c) k -> p c k", p=P)
    i_view = i_out.ap().rearrange("(p c) k -> p c k", p=P)

    with TileContext(nc) as tc:
        with (
            tc.tile_pool(name="const", bufs=1) as cpool,
            tc.tile_pool(name="xq", bufs=2) as xpool,
            tc.tile_pool(name="psum", bufs=2, space="PSUM") as ppool,
            tc.tile_pool(name="post", bufs=2) as spool,
        ):
            wh_sb = cpool.tile([P, kd, E], f16)
            nc.sync.dma_start(wh_sb[:], wh_d.ap())
            wl_sb = cpool.tile([P, kd, E], f16)
            nc.sync.dma_start(wl_sb[:], wl_d.ap())
            br_sb = cpool.tile([P, E], f32)
            nc.sync.dma_start(br_sb[:], br_d.ap())
            ident = cpool.tile([P, P], f32)
            make_identity(nc, ident[:])

            max_tiles = max(group_tiles)
            c0 = 0
            for q, g_tiles in enumerate(group_tiles):
                n_tok = g_tiles * P
                xsrc = xc_ds[q].ap()
                xq = xpool.tile([P, kd, 2, n_tok], f16, tag="xq")
                # split big group loads so the first matmuls start earlier
                n_dma = 2 if g_tiles >= 4 else 1
                step = kd // n_dma
                for j in range(n_dma):
                    nc.sync.dma_start(
                        xq[:, j * step : (j + 1) * step],
                        xsrc[:, j * step : (j + 1) * step],
                    )

                psA_t = ppool.tile([P, n_tok], f32, tag="psA", name=f"psA_{q}")
                psB_t = ppool.tile([P, n_tok], f32, tag="psB", name=f"psB_{q}")
                psA, psB = psA_t[:], psB_t[:]
                for ko2 in range(pairs):
                    k0, k1 = 2 * ko2, 2 * ko2 + 1
                    first = ko2 == 0
                    last = ko2 == pairs - 1
                    xh0, xl0 = xq[:, k0, 0, :], xq[:, k0, 1, :]
                    xh1, xl1 = xq[:, k1, 0, :], xq[:, k1, 1, :]
                    nc.tensor.matmul(
                        psA[0:64, :], wh_sb[:, k0, :], xh0,
                        tile_position=(0, 0), start=first, stop=last,
                        skip_group_check=True,
                    )
                    nc.tensor.matmul(
                        psA[64:128, :], wh_sb[:, k1, :], xh1,
                        tile_position=(0, 64), start=first, stop=last,
                        skip_group_check=True,
                    )
                    nc.tensor.matmul(
                        psB[0:64, :], wh_sb[:, k0, :], xl0,
                        tile_position=(0, 0), start=first, stop=False,
                        skip_group_check=True,
                    )
                    nc.tensor.matmul(
                        psB[64:128, :], wh_sb[:, k1, :], xl1,
                        tile_position=(0, 64), start=first, stop=False,
                        skip_group_check=True,
                    )
                    nc.tensor.matmul(
                        psB[0:64, :], wl_sb[:, k0, :], xh0,
                        tile_position=(0, 0), start=False, stop=last,
                        skip_group_check=True,
                    )
                    nc.tensor.matmul(
                        psB[64:128, :], wl_sb[:, k1, :], xh1,
                        tile_position=(0, 64), start=False, stop=last,
                        skip_group_check=True,
                    )

                # partial logits (chunk-parity halves stacked on partitions):
                # logitTT = psA + 2^-12 * psB    [128, n_tok]
                psA_sb = spool.tile([P, n_tok], f32, tag="psA_sb")
                nc.vector.tensor_copy(psA_sb[:], psA)
                logitTT = spool.tile([P, n_tok], f32, tag="logitTT")
                nc.vector.scalar_tensor_tensor(
                    logitTT[:], psB, LO_SCALE, psA_sb[:],
                    op0=mybir.AluOpType.mult, op1=mybir.AluOpType.add,
                )
                # transpose to token-major: psT [128 tok, (half, 64 expert)]
                psT_t = ppool.tile([P, n_tok], f32, tag="psT", name=f"psT_{q}")
                for i in range(g_tiles):
                    nc.tensor.transpose(
                        psT_t[:, i * P : (i + 1) * P],
                        logitTT[:, i * P : (i + 1) * P],
                        ident[:],
                    )
                cc = spool.tile([P, g_tiles, P], f32, tag="cc")
                nc.vector.tensor_copy(
                    cc[:],
                    psT_t[:].rearrange("p (c he) -> p c he", c=g_tiles),
                )
                # sum the parity halves, add bias, sigmoid
                logit_q = spool.tile([P, g_tiles, E], f32, tag="logit_q")
                nc.vector.tensor_add(logit_q[:], cc[:, :, 0:E], cc[:, :, E:P])
                nc.vector.tensor_add(
                    logit_q[:],
                    logit_q[:],
                    br_sb[:, None, :].to_broadcast([P, g_tiles, E]),
                )
                s_q = spool.tile([P, g_tiles, E], f32, tag="s_q")
                nc.scalar.activation(
                    s_q[:], logit_q[:], mybir.ActivationFunctionType.Sigmoid
                )

                # top-8 + normalize
                gmax = spool.tile([P, g_tiles, TOPK], f32, tag="gmax")
                gidx = spool.tile([P, g_tiles, TOPK], mybir.dt.uint32, tag="gidx")
                gsum = spool.tile([P, g_tiles], f32, tag="gsum")
                grec = spool.tile([P, g_tiles], f32, tag="grec")
                gnrm = spool.tile([P, g_tiles, TOPK], f32, tag="gnrm")
                for i in range(g_tiles):
                    nc.vector.max(out=gmax[:, i], in_=s_q[:, i])
                    nc.vector.max_index(
                        out=gidx[:, i], in_max=gmax[:, i], in_values=s_q[:, i]
                    )
                nc.vector.reduce_sum(gsum[:], gmax[:], axis=mybir.AxisListType.X)
                nc.vector.reciprocal(grec[:], gsum[:])
                nc.vector.tensor_mul(
                    gnrm[:],
                    gmax[:],
                    grec[:, :, None].to_broadcast([P, g_tiles, TOPK]),
                )

                c1 = c0 + g_tiles
                nc.sync.dma_start(s_view[:, c0:c1, :], s_q[:])
                nc.sync.dma_start(g_view[:, c0:c1, :], gnrm[:])
                nc.sync.dma_start(
                    i_view[:, c0:c1, :], gidx[:].bitcast(mybir.dt.int32)
                )
                c0 = c1

    nc.compile()
    return nc


def _get_nc():
    key = "main"
    if key not in _CACHE:
        _CACHE[key] = _build_kernel()
    return _CACHE[key]


def _split_hl(a32):
    """a32 (fp32) -> (hi fp16, lo fp16) with a32 ~= hi + 2^-12 * lo."""
    hi = a32.astype(np.float16)
    lo = ((a32 - hi.astype(np.float32)) * 4096.0).astype(np.float16)
    return hi, lo


def _pack_x(x, group_tiles, chunks, kd):
    """x [cores, tok_per_core, d] fp32 -> list of per-group packed fp16
    arrays [cores, P, kd, 2, n_tok].

    Group g covers token column-chunks [c0, c0+tiles); within it, matmul
    column i*128+pp holds token pp*chunks + c0 + i.
    """
    n_cores = x.shape[0]
    xh, xl = _split_hl(x.reshape(n_cores, P, chunks, kd, P))
    parts = []
    c0 = 0
    for g_tiles in group_tiles:
        # [core, pp, tiles, ko, p] -> [core, p, ko, tiles, pp]
        gh = xh[:, :, c0 : c0 + g_tiles].transpose(0, 4, 3, 2, 1)
        gl = xl[:, :, c0 : c0 + g_tiles].transpose(0, 4, 3, 2, 1)
        blk = np.empty((n_cores, P, kd, 2, g_tiles, P), np.float16)
        blk[:, :, :, 0] = gh
        blk[:, :, :, 1] = gl
        parts.append(blk.reshape(n_cores, P, kd, 2, g_tiles * P))
        c0 += g_tiles
    return parts


def _prep_inputs(x, W, b):
    x = np.asarray(x, dtype=np.float32)
    W = np.ascontiguousarray(np.asarray(W, dtype=np.float32))
    b = np.ascontiguousarray(np.asarray(b, dtype=np.float32))

    xc_parts = _pack_x(x.reshape(N_CORES, TOK_PER_CORE, D), GROUP_TILES, CHUNKS, KD)

    wh, wl = _split_hl(W)
    wh_sb = np.ascontiguousarray(wh.reshape(KD, P, E).transpose(1, 0, 2))
    wl_sb = np.ascontiguousarray(wl.reshape(KD, P, E).transpose(1, 0, 2))
    br = np.ascontiguousarray(np.broadcast_to(b.reshape(1, E), (P, E)))

    in_maps = []
    for j in range(N_CORES):
        m = {"wh": wh_sb, "wl": wl_sb, "br": br}
        for q, part in enumerate(xc_parts):
            m[f"xc{q}"] = part[j]
        in_maps.append(m)
    return in_maps


def kernel(x, W, b, k):
    global LAST_RESULTS
    from concourse.bass_utils import run_bass_kernel_spmd

    k = int(np.asarray(k))
    assert k == TOPK, f"kernel hardcodes k=8, got {k}"
    assert tuple(np.asarray(x).shape) == (B, S, D)

    nc = _get_nc()
    in_maps = _prep_inputs(x, W, b)

    res = run_bass_kernel_spmd(
        nc,
        in_maps,
        core_ids=list(range(N_CORES)),
        trace=bool(int(os.environ.get("KERNEL_TRACE", "0"))),
    )
    LAST_RESULTS = res

    g = np.concatenate([r["g_out"] for r in res.results], axis=0)
    idx = np.concatenate([r["i_out"] for r in res.results], axis=0)
    s = np.concatenate([r["s_out"] for r in res.results], axis=0)
    return (
        g.reshape(B, S, TOPK),
        idx.reshape(B, S, TOPK).astype(np.int32),
        s.reshape(B, S, E),
    )
